# revision 5
# baseline (speedup 1.0000x reference)
"""Trilinear 2x upsampling (TF v1 asymmetric coords) on 8 Trainium2 cores.

Math: for each resize axis, out[2i] = in[i] and out[2i+1] = 0.5*(in[i] +
in[i+1]) (edge-clamped).  The 3D op separates into 8 (H,W,D)-parity classes,
each a product of copies/neighbor-averages.

Sharding: input [2,96,96,48,32] -> [64 BC, 96 H, 96 W, 48 D].  SBUF partition
p = half*64 + bc where half splits H in two 48-row blocks: 128 partitions.
Each of the 8 cores owns 6 input H-rows per partition (+1 halo row), i.e.
12 consecutive output H-rows.  All averaging runs along free dims (W, D and
the in-window H rows), so no cross-partition ops are needed.

Per core the device emits y[128, 12, 192, 96]: fully W/D-interleaved output
planes (even-H plane = W/D-upsample of row r; odd-H plane = H-average).  The
host only stacks per-core results and transposes back to channels-last.
"""

import sys
import numpy as np

for _p in ("/opt/trn_rl_repo",):
    if _p not in sys.path:
        sys.path.insert(0, _p)

import concourse.mybir as mybir  # noqa: E402
from concourse import bass, tile  # noqa: E402
from concourse import bass_utils  # noqa: E402

F32 = mybir.dt.float32

B, C, H, W, D = 2, 32, 96, 96, 48
TH, TW, TD = 192, 192, 96
NCORES = 8
ROWS = 6        # owned input H rows per (core, half)
WQ = 24         # input W cols per quarter-step
NQ = W // WQ    # 4

_ws_ctr = [0]


def _split_multi_waits(nc):
    """The walrus in this environment accepts at most one semaphore wait per
    instruction (two on EventSemaphore).  Tile's wait assigner can attach
    more; move the extras onto EventSemaphore instructions inserted just
    before, on the same engine, preserving program order."""
    n_split = 0
    for f in nc.m.functions:
        for blk in f.blocks:
            out = []
            changed = False
            for inst in blk.instructions:
                si = inst.sync_info
                waits = list(si.on_wait) if si and si.on_wait else []
                cap = 2 if isinstance(inst, mybir.InstEventSemaphore) else 1
                if len(waits) > cap:
                    changed = True
                    n_split += 1
                    extra = waits[:-1]
                    for i in range(0, len(extra), 2):
                        _ws_ctr[0] += 1
                        ev = mybir.InstEventSemaphore(
                            name=f"ws_ev_{_ws_ctr[0]}", ins=[], outs=[])
                        ev.engine = inst.engine
                        ev.sync_info = mybir.SyncInfo(
                            on_wait=list(extra[i:i + 2]), on_update=[])
                        out.append(ev)
                    si.on_wait = [waits[-1]]
                    inst.sync_info = si
                out.append(inst)
            if changed:
                blk.instructions = out
    return n_split


def build_program():
    nc = bass.Bass()
    x = nc.dram_tensor("x", [128, ROWS + 1, W + 1, D], F32,
                       kind="ExternalInput")
    y = nc.dram_tensor("y", [128, 2 * ROWS, TW, TD], F32,
                       kind="ExternalOutput")

    WO = 2 * WQ  # output W cols per step (48)

    with tile.TileContext(nc) as tc:
        with tc.tile_pool(name="pool", bufs=2) as pool:
            for q in range(NQ):
                prev = None
                for r in range(ROWS + 1):
                    owned = r < ROWS
                    # per-row input window load (614 KB per DMA)
                    p0 = pool.tile([128, WQ + 1, D], F32, tag="p0", bufs=4,
                                   name=f"p0_{q}_{r}")
                    nc.sync.dma_start(out=p0,
                                      in_=x[:, r, q * WQ: q * WQ + WQ + 1, :])
                    # ph = 0.5 * input row (halved operand for every average)
                    ph = pool.tile([128, WQ + 1, D], F32, tag="ph", bufs=4,
                                   name=f"ph_{q}_{r}")
                    nc.scalar.mul(ph, p0, 0.5)

                    # me: even-H output plane [WO, TD], W/D interleaved
                    me = pool.tile([128, WO, TD], F32, tag="me", bufs=3,
                                   name=f"me_{q}_{r}")
                    if owned:
                        # (e,e,e): copy of input
                        nc.scalar.copy(me[:, 0:WO:2, 0:TD:2], p0[:, 0:WQ, :])
                        # (e,e,o) D-edge: clamp -> copy
                        nc.scalar.copy(me[:, 0:WO:2, TD - 1], p0[:, 0:WQ, D - 1])
                    # (e,e,o): B = avg along D
                    nc.vector.tensor_add(me[:, 0:WO:2, 1:TD - 1:2],
                                         ph[:, 0:WQ, 0:D - 1], ph[:, 0:WQ, 1:D])
                    # B at the W-halo column (feeds bh only)
                    bha = pool.tile([128, D], F32, tag="bha", bufs=4,
                                    name=f"bha_{q}_{r}")
                    nc.vector.tensor_add(bha[:, 0:D - 1],
                                         ph[:, WQ, 0:D - 1], ph[:, WQ, 1:D])
                    # (e,o,e): Ce = avg along W
                    nc.vector.tensor_add(me[:, 1:WO:2, 0:TD:2],
                                         ph[:, 0:WQ, :], ph[:, 1:WQ + 1, :])
                    # bh = 0.5 * B  (including W-halo col and D-edge col)
                    bh = pool.tile([128, WQ + 1, D], F32, tag="bh", bufs=4,
                                   name=f"bh_{q}_{r}")
                    nc.scalar.mul(bh[:, 0:WQ, 0:D - 1],
                                  me[:, 0:WO:2, 1:TD - 1:2], 0.5)
                    nc.scalar.mul(bh[:, WQ, 0:D - 1], bha[:, 0:D - 1], 0.5)
                    nc.scalar.copy(bh[:, :, D - 1], ph[:, :, D - 1])
                    # (e,o,o): Cd = avg along W of B
                    nc.vector.tensor_add(me[:, 1:WO:2, 1:TD:2],
                                         bh[:, 0:WQ, :], bh[:, 1:WQ + 1, :])
                    # halved planes feeding the H-averages
                    ceh = pool.tile([128, WQ, D], F32, tag="ceh", bufs=4,
                                    name=f"ceh_{q}_{r}")
                    nc.scalar.mul(ceh, me[:, 1:WO:2, 0:TD:2], 0.5)
                    cdh = pool.tile([128, WQ, D], F32, tag="cdh", bufs=4,
                                    name=f"cdh_{q}_{r}")
                    nc.scalar.mul(cdh, me[:, 1:WO:2, 1:TD:2], 0.5)

                    cur = dict(me=me, ph=ph, bh=bh, ceh=ceh, cdh=cdh)
                    if prev is not None:
                        rr = r - 1
                        # mo: odd-H output plane = H-average of the W/D
                        # upsampled rows rr and rr+1
                        mo = pool.tile([128, WO, TD], F32, tag="mo", bufs=3,
                                       name=f"mo_{q}_{rr}")
                        nc.vector.tensor_add(mo[:, 0:WO:2, 0:TD:2],
                                             prev["ph"][:, 0:WQ, :],
                                             ph[:, 0:WQ, :])
                        nc.vector.tensor_add(mo[:, 0:WO:2, 1:TD:2],
                                             prev["bh"][:, 0:WQ, :],
                                             bh[:, 0:WQ, :])
                        nc.vector.tensor_add(mo[:, 1:WO:2, 0:TD:2],
                                             prev["ceh"], ceh)
                        nc.vector.tensor_add(mo[:, 1:WO:2, 1:TD:2],
                                             prev["cdh"], cdh)
                        nc.sync.dma_start(
                            out=y[:, 2 * rr, q * WO:(q + 1) * WO, :],
                            in_=prev["me"])
                        nc.sync.dma_start(
                            out=y[:, 2 * rr + 1, q * WO:(q + 1) * WO, :],
                            in_=mo)
                    prev = cur

    _split_multi_waits(nc)
    return nc


def _prep_inputs(x):
    """Full [2,96,96,48,32] fp32 -> per-core in_maps [128, 7, 97, 48]."""
    xt = np.transpose(x, (0, 4, 1, 2, 3)).reshape(B * C, H, W, D)
    # pad W by one edge-replicated column
    xp = np.concatenate([xt, xt[:, :, W - 1:W, :]], axis=2)  # [64,96,97,48]
    in_maps = []
    for k in range(NCORES):
        parts = []
        for half in (0, 1):
            rows = np.minimum(half * 48 + k * ROWS + np.arange(ROWS + 1), H - 1)
            parts.append(xp[:, rows])  # [64, 7, 97, 48]
        xin = np.stack(parts, axis=0).reshape(128, ROWS + 1, W + 1, D)
        in_maps.append({"x": np.ascontiguousarray(xin)})
    return in_maps


def _assemble(results):
    """Per-core y [128,12,192,96] -> full [2,192,192,96,32]."""
    big = np.empty((B * C, 2, NCORES, 2 * ROWS, TW, TD), np.float32)
    for k in range(NCORES):
        yk = np.asarray(results[k]["y"]).reshape(2, B * C, 2 * ROWS, TW, TD)
        big[:, 0, k] = yk[0]
        big[:, 1, k] = yk[1]
    out_bc = big.reshape(B * C, TH, TW, TD)
    out = out_bc.reshape(B, C, TH, TW, TD).transpose(0, 2, 3, 4, 1)
    return np.ascontiguousarray(out)


def kernel(x, _trace=False):
    x = np.ascontiguousarray(np.asarray(x), dtype=np.float32)
    assert x.shape == (B, H, W, D, C), x.shape
    in_maps = _prep_inputs(x)
    nc = build_program()
    kw = {}
    if _trace:
        kw = dict(trace=True)
    res = bass_utils.run_bass_kernel_spmd(
        nc, in_maps, core_ids=list(range(NCORES)), **kw)
    out = _assemble(res.results)
    if _trace:
        return out, res
    return out


if __name__ == "__main__":
    rng = np.random.default_rng(0)
    x = rng.standard_normal((B, H, W, D, C), dtype=np.float32)
    y = kernel(x)
    print("out shape:", y.shape, y.dtype)
